# revision 4
# baseline (speedup 1.0000x reference)
"""ChannelDisassembly Trainium kernel.

Splits "outlier" channels (per-channel max|x| > 8) into T = ceil(max/8)
identical copies scaled by 1/T.  The replication plan is computed host-side
from the global per-channel maxima (mirrors the reference's host sync), then
a Bass/Tile kernel applies the gather+scale, data-parallel over the batch
dim across 8 NeuronCores.

Per core: read its [B/8, C, H*W] batch shard, scale each outlier channel by
1/T on DVE, and DMA each scaled channel back out T times (single DMA per
channel tile using a broadcast source AP).
"""

import numpy as np

THRESHOLD = 8.0
B, C, H, W = 16, 512, 56, 56
HW = H * W
N_CORES = 8
BSH = B // N_CORES  # batches per core
P = 128  # SBUF partitions

PROFILE = False  # test harness sets True to collect exec_time_ns
LAST_EXEC_NS = None
LAST_RESULTS = None

_build_cache = {}


def _plan(x):
    max_vals = np.max(np.abs(x), axis=(0, 2, 3))
    outlier = np.nonzero(max_vals > THRESHOLD)[0]
    T = np.ceil(max_vals[outlier] / THRESHOLD).astype(np.int64)
    return outlier, T


def _tile_runs(outlier, T, starts, i, n_out):
    """Contiguous-source load runs and equal-T store runs for channel tile i."""
    lo, hi = i * P, min((i + 1) * P, n_out)
    chans = outlier[lo:hi]
    loads = []  # (dst_partition, src_channel, run_len)
    rs = 0
    for k in range(1, len(chans) + 1):
        if k == len(chans) or chans[k] != chans[k - 1] + 1:
            loads.append((rs, int(chans[rs]), k - rs))
            rs = k
    stores = []  # (partition0, run_len, T, out_channel_start)
    ts, st = T[lo:hi], starts[lo:hi]
    rs = 0
    for k in range(1, len(chans) + 1):
        if k == len(chans) or ts[k] != ts[rs]:
            stores.append((rs, k - rs, int(ts[rs]), int(st[rs])))
            rs = k
    return loads, stores, hi - lo


def _build(outlier, T):
    import concourse.bacc as bacc
    import concourse.mybir as mybir
    import concourse.tile as tile

    f32 = mybir.dt.float32
    n_out = len(outlier)
    n_tiles = (n_out + P - 1) // P
    starts = np.concatenate([[0], np.cumsum(T)[:-1]]).astype(np.int64)
    R = int(T.sum())

    nc = bacc.Bacc(None)
    x_d = nc.declare_dram_parameter("x", [BSH * C, HW], f32, isOutput=False)
    s_d = nc.declare_dram_parameter("s", [P, n_tiles], f32, isOutput=False)
    y_d = nc.declare_dram_parameter("y", [BSH * R, HW], f32, isOutput=True)

    with tile.TileContext(nc) as tc:
        with (
            tc.tile_pool(name="sc", bufs=1) as scp,
            tc.tile_pool(name="io", bufs=6) as iop,
        ):
            sc = scp.tile([P, n_tiles], f32)
            nc.sync.dma_start(sc[:], s_d[:])
            for b in range(BSH):
                for i in range(n_tiles):
                    loads, stores, p = _tile_runs(outlier, T, starts, i, n_out)
                    t = iop.tile([P, HW], f32)
                    for dp, src_c, ln in loads:
                        nc.sync.dma_start(
                            t[dp : dp + ln, :],
                            x_d[b * C + src_c : b * C + src_c + ln, :],
                        )
                    nc.vector.tensor_scalar_mul(t[:p, :], t[:p, :], sc[:p, i : i + 1])
                    for p0, ln, tt, os_ in stores:
                        dst = y_d[
                            b * R + os_ : b * R + os_ + ln * tt, :
                        ].rearrange("(l t) f -> l t f", t=tt)
                        src = t[p0 : p0 + ln, :].unsqueeze(1).broadcast_to(
                            [ln, tt, HW]
                        )
                        nc.scalar.dma_start(dst, src)
    nc.finalize()  # Bacc: split multi-wait instructions, allocate registers
    return nc


def kernel(x):
    global LAST_EXEC_NS, LAST_RESULTS
    from concourse.bass_utils import run_bass_kernel_spmd

    x = np.ascontiguousarray(np.asarray(x), dtype=np.float32)
    assert x.shape == (B, C, H, W), x.shape

    outlier, T = _plan(x)
    R = int(T.sum())
    if R == 0:
        return (
            np.zeros((B, 0, H, W), np.float32),
            outlier.astype(np.int32),
        )

    key = (outlier.tobytes(), T.tobytes())
    if key not in _build_cache:
        _build_cache[key] = _build(outlier, T)
    nc = _build_cache[key]

    n_out = len(outlier)
    n_tiles = (n_out + P - 1) // P
    inv_T = (1.0 / T).astype(np.float32)
    svec = np.ones((P, n_tiles), np.float32)
    for j in range(n_out):
        svec[j % P, j // P] = inv_T[j]

    xs = x.reshape(B, C, HW)
    in_maps = [
        {"x": np.ascontiguousarray(xs[c * BSH : (c + 1) * BSH].reshape(BSH * C, HW)),
         "s": svec}
        for c in range(N_CORES)
    ]
    res = run_bass_kernel_spmd(
        nc, in_maps, core_ids=list(range(N_CORES)), trace=PROFILE
    )
    LAST_EXEC_NS = res.exec_time_ns
    LAST_RESULTS = res
    out = np.concatenate(
        [r["y"].reshape(BSH, R, H, W) for r in res.results], axis=0
    )
    return out, outlier.astype(np.int32)
